# revision 35
# baseline (speedup 1.0000x reference)
"""Trainium2 Bass kernel for pairwise Tang-Toennies dispersion energy.

Problem: for P=3.2M random atom pairs over N=100k atoms in a periodic box,
    ene[p] = -(c6[ti,tj] * f6(b[ti,tj]*r)) / r^6   if r <= cutoff else 0
with r the minimum-image distance and f6 the 6th-order Tang-Toennies damper.

Indirect DMA on TRN2 honours one dynamic offset per SBUF partition per
instruction (128 gathered rows/instruction) - far too slow for 6.4M row
fetches - so per-pair gathers are avoided with a degree-grouped layout:

  host:   group pairs by endpoint into fixed-size slot rows of a "virtual
          row" list.  All index math (sorting, slot maps, the A->B slot
          permutation) is host-side numpy; the host only routes bytes.
  pass 1: (j-grouped, R_A=8 slots/row) device expands the replicated atom
          coordinate table into one 12B record (x,y,z in box fractions)
          per j-slot via a 0-stride broadcast copy - pure streaming.
  host:   permutes pass-1 records from j-slot order to i-slot order.
  pass 2: rows are keyed by (i-atom, j-TYPE) with R_B=4 slots, so the
          c6/b pair coefficients are per-row window constants - no per-slot
          table select at all.  Minimum image via the magic-constant
          round-to-nearest trick (matches the reference's round()), all in
          box-fraction coordinates with the box scale folded into the
          activation scale/bias.  r and 1/r^6 come from one Ln + two Exp.
  host:   scatters per-slot energies back to the original pair order.

Engine budget per [128 x 640] tile op (CoreSim cost model): DVE ts/stt
394ns, DVE tensor_tensor 727ns, Act activation 718ns, Pool ts/stt 984ns.
Every 2-tensor op is therefore emitted as scalar_tensor_tensor (stt) and
work is spread across DVE/Pool/Act to balance busy time.  All activation
functions used (Copy/Square/Ln/Exp) live in the single hardware table set
'natural_log_exp_and_others'; the other table sets presented to the
act-table-load placement pass are stripped of the shared functions so a
single hoisted table load serves the whole kernel (no per-tile reloads).
"""

import numpy as np

import concourse.bacc as bacc
import concourse.bass as bass
import concourse.mybir as mybir
from concourse.tile import TileContext
from concourse.bass_utils import run_bass_kernel_spmd

F32 = mybir.dt.float32
U8 = mybir.dt.uint8
AF = mybir.ActivationFunctionType
OP = mybir.AluOpType

N_CORES = 8

# pass 1 (A side, grouped by j atom)
R_A = 8
WA1 = 80
K1 = WA1 * R_A            # 640 slots / partition / tile
TV1 = 128 * WA1           # 10240 virtual rows / tile

# pass 2 (B side, grouped by (i atom, j type))
R_B = 2
WA2 = 320
K2 = WA2 * R_B            # 640 slots / partition / tile
TV2 = 128 * WA2           # 40960 virtual rows / tile

MAGIC = 12582912.0        # 1.5 * 2**23: float32 round-to-nearest-even trick


# ---------------------------------------------------------------------------
# Pin the activation table: every function we use (Copy/Square/Ln/Exp) is in
# 'natural_log_exp_and_others'.  The load-placement pass picks tables
# greedily by list order, which thrashes between exp/ln-only sets; stripping
# the shared functions from every OTHER set (names and indices unchanged, so
# act_func_set_id stays canonical for walrus) forces the single covering set.
# ---------------------------------------------------------------------------
from concourse.hw_specs import get_activation_tables as _get_act_tables

_PIN = "natural_log_exp_and_others"


def _pinned_act_tables(arch):
    tabs = _get_act_tables(arch)
    pin = tabs[_PIN]
    return {n: (s if n == _PIN else s - pin) for n, s in tabs.items()}


bacc.get_activation_tables = _pinned_act_tables


def build_pass1(vrows_core, reps=1, t_limit=None):
    """Expand slim rows [v,3] into AoS records [v*R_A, 3] (row repeated
    R_A times).  The expansion runs on DVE via a 0-stride read; both DMAs
    are dense."""
    import contextlib
    nc = bacc.Bacc(trn_type="TRN2", target_bir_lowering=False)
    slimv = nc.dram_tensor("slimv", [vrows_core, 3], F32, kind="ExternalInput")
    recs = nc.dram_tensor("recs", [vrows_core * R_A, 3], F32,
                          kind="ExternalOutput")
    T = vrows_core // TV1
    if t_limit is not None:
        T = min(T, t_limit)
    with TileContext(nc) as tc:
        rep_ctx = tc.For_i(0, reps, 1) if reps > 1 else contextlib.nullcontext()
        with tc.tile_pool(name="w", bufs=3) as pool, rep_ctx:
            for t in range(T):
                w = pool.tile([128, WA1 * 3], F32, tag="w")
                nc.sync.dma_start(
                    out=w[:],
                    in_=slimv[bass.ts(t, TV1)].rearrange(
                        "(p a) d -> p (a d)", a=WA1),
                )
                wp = w[:].ap[0]
                e = pool.tile([128, K1 * 3], F32, tag="e")
                nc.vector.tensor_copy(
                    e[:].rearrange("p (a r c) -> p a r c", a=WA1, r=R_A),
                    bass.AP(w.tensor, 0, [wp, [3, WA1], [0, R_A], [1, 3]]),
                )
                nc.sync.dma_start(
                    out=recs[bass.ts(t, TV1 * R_A)].rearrange(
                        "(p x) c -> p (x c)", x=K1),
                    in_=e[:],
                )
    nc.compile()
    return nc


def build_pass2(vrows_core, inv_ls, neg_ls, cf2, reps=1, t_limit=None):
    """(i-atom, j-type)-grouped energy kernel.

    ilite rows: x, y, z, c6_ij, b_ij, c6_ij*b_ij^6.
    jrec: per-slot records x, y, z of the j endpoint (routed by host).
    Minimum image: m = d - L*round(d/L), with round done by the magic
    float32 constant; the t' = d/L + MAGIC step runs as one Act Copy with
    scale+bias.  poly6 is a Horner chain v_k = (v_{k+1} + 1/k!)*u - one stt
    per degree.  The u<=1.2 Tang-Toennies branch uses
    es = -c6 b^6 * e^-u * (u*S'(u)) (the u^7/r^6 collapses into the
    per-row c6*b^6), so it needs no reciprocal powers at all.
    cf2 = cutoff**2 (absolute units).
    """
    import contextlib
    nc = bacc.Bacc(trn_type="TRN2", target_bir_lowering=False)
    # window coords pre-replicated R_B times: [x y z]*R_B per row (walrus
    # limits stt APs to 3D, so the broadcast must be a dense stride-1 dim)
    wxyz = nc.dram_tensor("wxyz", [vrows_core, 3 * R_B], F32,
                          kind="ExternalInput")
    wpar = nc.dram_tensor("wpar", [vrows_core, 3], F32, kind="ExternalInput")
    jrec = nc.dram_tensor("jrec", [vrows_core * R_B, 3], F32,
                          kind="ExternalInput")
    ene_d = nc.dram_tensor("ene", [vrows_core * R_B], F32,
                           kind="ExternalOutput")
    T = vrows_core // TV2
    if t_limit is not None:
        T = min(T, t_limit)
    # u*S'(u) = s0 u + s1 u^2 + s2 u^3 + s3 u^4 with S' = S/5040,
    # S = 1 + u/8 + u^2/72 + u^3/720 (truncation error < 3e-4 at u=1.2).
    s0, s1, s2, s3 = 1 / 5040., 1 / (8 * 5040.), 1 / (72 * 5040.), 1 / (720 * 5040.)
    C = [1.0, 1.0, 1 / 2., 1 / 6., 1 / 24., 1 / 120., 1 / 720.]  # 1/k!

    with TileContext(nc) as tc:
        rep_ctx = tc.For_i(0, reps, 1) if reps > 1 else contextlib.nullcontext()
        with tc.tile_pool(name="io", bufs=3) as pio, \
             tc.tile_pool(name="geo", bufs=2) as pgeo, \
             tc.tile_pool(name="long", bufs=3) as plong, \
             tc.tile_pool(name="work", bufs=2) as pool, rep_ctx:
            # Output stores are emitted two iterations late so the SP queue
            # never stalls on a not-yet-computed eout in front of the loads.
            pending = []
            for t in range(T):
                wx = pgeo.tile([128, WA2 * 3 * R_B], F32, tag="wx")
                nc.sync.dma_start(
                    out=wx[:],
                    in_=wxyz[bass.ts(t, TV2)].rearrange(
                        "(p a) d -> p (a d)", a=WA2),
                )
                w = pio.tile([128, WA2 * 3], F32, tag="w")
                nc.sync.dma_start(
                    out=w[:],
                    in_=wpar[bass.ts(t, TV2)].rearrange(
                        "(p a) d -> p (a d)", a=WA2),
                )
                jt = pgeo.tile([128, K2 * 3], F32, tag="jrec")
                nc.sync.dma_start(
                    out=jt[:],
                    in_=jrec[bass.ts(t, TV2 * R_B)].rearrange(
                        "(p x) c -> p (x c)", x=K2),
                )
                if len(pending) >= 2:
                    pending.pop(0)()
                wp = w[:].ap[0]
                wxp = wx[:].ap[0]

                def wbc(off):
                    return bass.AP(w.tensor, off, [wp, [3, WA2], [0, R_B]])

                def t3(x):
                    return x[:].rearrange("p (a r) -> p a r", a=WA2)

                def tw(x):
                    # [p, WA2, 12] view of a [128, K2*3] tile
                    return x[:].rearrange("p (a x) -> p a x", a=WA2)

                def stt(eng, out, in0, scalar, in1, op0, op1):
                    eng.scalar_tensor_tensor(out=out, in0=in0, scalar=scalar,
                                             in1=in1, op0=op0, op1=op1)

                # ---- minimum image (round via magic constant), r2 ----
                # all 3 axes in one instruction per step (cubic box)
                d_all = pool.tile([128, K2 * 3], F32, tag="d_all")
                nc.gpsimd.tensor_tensor(
                    out=tw(d_all), in0=tw(jt),
                    in1=bass.AP(wx.tensor, 0,
                                [wxp, [3 * R_B, WA2], [1, 3 * R_B]]),
                    op=OP.subtract)
                t_all = pool.tile([128, K2 * 3], F32, tag="t_all")
                nc.scalar.activation(t_all[:], d_all[:], AF.Copy,
                                     scale=float(inv_ls[0]), bias=MAGIC)
                rd_all = pool.tile([128, K2 * 3], F32, tag="rd_all")
                nc.vector.tensor_scalar(
                    out=rd_all[:], in0=t_all[:], scalar1=MAGIC, scalar2=None,
                    op0=OP.subtract)
                m_all = pool.tile([128, K2 * 3], F32, tag="m_all")
                stt(nc.vector, m_all[:], rd_all[:], float(neg_ls[0]),
                    d_all[:], OP.mult, OP.add)
                # q reuses the d_all rotation (d is dead once m exists)
                q_all = pool.tile([128, K2 * 3], F32, tag="d_all")
                nc.scalar.activation(q_all[:], m_all[:], AF.Square)
                r2 = plong.tile([128, K2], F32, tag="r2")
                nc.vector.tensor_reduce(
                    out=r2[:], in_=q_all[:].rearrange("p (k c) -> p k c", c=3),
                    axis=mybir.AxisListType.X, op=OP.add)

                # ---- r, 1/r^6, u, e^-u ----
                lr = pool.tile([128, K2], F32, tag="lr")
                nc.scalar.activation(lr[:], r2[:], AF.Ln)
                rr = pool.tile([128, K2], F32, tag="rr")
                nc.scalar.activation(rr[:], lr[:], AF.Exp, scale=0.5)
                ir6 = pool.tile([128, K2], F32, tag="ir6")
                nc.scalar.activation(ir6[:], lr[:], AF.Exp, scale=-3.0)
                u = plong.tile([128, K2], F32, tag="u")
                nc.gpsimd.tensor_tensor(out=t3(u), in0=t3(rr), in1=wbc(1),
                                        op=OP.mult)
                em = plong.tile([128, K2], F32, tag="em")
                nc.scalar.activation(em[:], u[:], AF.Exp, scale=-1.0)

                A = plong.tile([128, K2], F32, tag="A")
                nc.gpsimd.tensor_tensor(out=t3(A), in0=t3(ir6), in1=wbc(0),
                                        op=OP.mult)

                # ---- poly6 Horner chain: v_k = (v_{k+1} + 1/k!) * u ----
                v = pool.tile([128, K2], F32, tag="v_a")
                nc.scalar.activation(v[:], u[:], AF.Copy, scale=C[6])
                for i, (k, eng) in enumerate(
                        ((5, nc.vector), (4, nc.vector), (3, nc.vector),
                         (2, nc.vector), (1, nc.vector))):
                    v2_ = pool.tile([128, K2], F32, tag=f"v_{'ba'[i % 2]}")
                    stt(eng, v2_[:], v[:], C[k], u[:], OP.add, OP.mult)
                    v = v2_

                # main = A*(em*poly6 - 1),  poly6 = 1 + v
                tpe = pool.tile([128, K2], F32, tag="tpe")
                nc.gpsimd.tensor_tensor(out=tpe[:], in0=v[:], in1=em[:],
                                        op=OP.mult)
                sm = pool.tile([128, K2], F32, tag="sm")
                stt(nc.vector, sm[:], em[:], -1.0, tpe[:], OP.add, OP.add)
                main = pool.tile([128, K2], F32, tag="main")
                nc.gpsimd.tensor_tensor(out=main[:], in0=sm[:], in1=A[:],
                                        op=OP.mult)

                # ---- small-u branch: es = -c6 b^6 em G(u), G = u S'(u) ----
                G = pool.tile([128, K2], F32, tag="G_a")
                nc.vector.tensor_scalar(
                    out=G[:], in0=u[:], scalar1=s3, scalar2=s2,
                    op0=OP.mult, op1=OP.add)
                for i, (cadd, eng) in enumerate(
                        ((None, nc.vector), (s1, nc.vector), (s0, nc.vector))):
                    G2 = pool.tile([128, K2], F32, tag=f"G_{'ba'[i % 2]}")
                    stt(eng, G2[:], G[:], 0.0 if cadd is None else cadd,
                        u[:], OP.add, OP.mult)
                    G = G2
                ta = pool.tile([128, K2], F32, tag="ta")
                nc.gpsimd.tensor_tensor(out=ta[:], in0=G[:], in1=em[:],
                                        op=OP.mult)
                es = pool.tile([128, K2], F32, tag="es")
                nc.gpsimd.tensor_tensor(out=t3(es), in0=t3(ta), in1=wbc(2),
                                        op=OP.mult)

                mu = pool.tile([128, K2], U8, tag="mu")
                nc.vector.tensor_scalar(
                    out=mu[:], in0=u[:], scalar1=1.2, scalar2=None, op0=OP.is_le)
                nc.vector.copy_predicated(main[:], mu[:], es[:])

                eout = pio.tile([128, K2], F32, tag="eout")
                stt(nc.vector, eout[:], r2[:], float(cf2), main[:],
                    OP.is_le, OP.mult)

                def mk_store(t=t, eout=eout):
                    nc.sync.dma_start(
                        out=ene_d[bass.ts(t, TV2 * R_B)].rearrange(
                            "(p x) -> p x", x=K2),
                        in_=eout[:])
                pending.append(mk_store)
            for fn in pending:
                fn()
    nc.compile()
    return nc


def _slot_map(key_of_pair, n_keys, R):
    """Return (slot per pair, virtual-row key ids, V) for one grouping."""
    P = key_of_pair.shape[0]
    d = np.bincount(key_of_pair, minlength=n_keys)
    rows = -(-d // R)  # ceil; 0 for empty keys
    vbase = np.zeros(n_keys + 1, np.int64)
    np.cumsum(rows, out=vbase[1:])
    V = int(vbase[-1])
    v_key = np.repeat(np.arange(n_keys, dtype=np.int64), rows)
    order = np.argsort(key_of_pair, kind="stable")
    pb = np.zeros(n_keys + 1, np.int64)
    np.cumsum(d, out=pb[1:])
    k_sorted = key_of_pair[order]
    rank = np.arange(P, dtype=np.int64) - pb[k_sorted]
    slot_sorted = (vbase[k_sorted] + rank // R) * R + rank % R
    slot = np.empty(P, np.int64)
    slot[order] = slot_sorted
    return slot, v_key, V


def _pad_vrows(V, tile_rows):
    per_core = -(-(V + 1) // (tile_rows * N_CORES)) * tile_rows
    return per_core * N_CORES, per_core


_NC_CACHE = {}


def _get_nc(builder, key, *args):
    if key not in _NC_CACHE:
        _NC_CACHE[key] = builder(*args)
    return _NC_CACHE[key]


def _host_reference(coords, pairs, box, c6, b, cutoff, atom_types):
    # numpy fallback for non-orthorhombic boxes (not hit by the real inputs)
    dr = coords[pairs[:, 1]] - coords[pairs[:, 0]]
    inv_box = np.linalg.inv(box)
    dr = dr - np.round(dr @ inv_box) @ box
    r = np.sqrt((dr * dr).sum(1))
    ti = atom_types[pairs[:, 0]]
    tj = atom_types[pairs[:, 1]]
    u = b[ti, tj] * r
    poly = 1.0 + u * (1.0 + u / 2.0 * (1.0 + u / 3.0 * (1.0 + u / 4.0 *
                     (1.0 + u / 5.0 * (1.0 + u / 6.0)))))
    f6 = 1.0 - np.exp(-u) * poly
    ene = -(c6[ti, tj] * f6) / r ** 6
    return np.where(r <= cutoff, ene, 0.0).astype(np.float32)


def _prepare(coords, pairs, box, c6, b, cutoff, atom_types):
    """All host-side index math + byte layout for both passes."""
    Ls = np.array([box[i, i] for i in range(3)], np.float64)
    n_atoms = coords.shape[0]
    pi = np.ascontiguousarray(pairs[:, 0]).astype(np.int64)
    pj = np.ascontiguousarray(pairs[:, 1]).astype(np.int64)
    ti = atom_types[pi]
    tj = atom_types[pj]
    n_types = c6.shape[0]

    # A side: grouped by j atom
    sA, vj_atom, Vj = _slot_map(pj, n_atoms, R_A)
    Vj_pad, vj_core = _pad_vrows(Vj, TV1)
    slimv = np.empty((Vj_pad, 3), np.float32)
    slimv[:Vj] = coords[vj_atom]
    slimv[Vj:] = 1e4

    # B side: grouped by (i atom, j type)
    keyB = pi * n_types + tj
    sB, v_key, Vi = _slot_map(keyB, n_atoms * n_types, R_B)
    Vi_pad, vi_core = _pad_vrows(Vi, TV2)
    ti_v = atom_types[v_key // n_types]
    tj_v = v_key % n_types
    c6b6 = (c6.astype(np.float64) * b.astype(np.float64) ** 6).astype(np.float32)
    wxyz = np.empty((Vi_pad, 3 * R_B), np.float32)
    wxyz[:Vi] = np.tile(coords[v_key // n_types], (1, R_B))
    wxyz[Vi:] = 2e4
    wpar = np.empty((Vi_pad, 3), np.float32)
    wpar[:Vi, 0] = c6[ti_v, tj_v]
    wpar[:Vi, 1] = b[ti_v, tj_v]
    wpar[:Vi, 2] = -c6b6[ti_v, tj_v]
    wpar[Vi:] = (0.0, 1.0, 0.0)

    a_of_b = np.full(Vi_pad * R_B, Vj_pad * R_A - 1, np.int64)
    a_of_b[sB] = sA

    inv_ls = tuple(float(np.float32(1.0) / np.float32(L)) for L in Ls)
    neg_ls = tuple(-float(L) for L in Ls)
    cf2 = float(np.float32(cutoff) ** 2)
    return dict(sB=sB, a_of_b=a_of_b, slimv=slimv, wxyz=wxyz, wpar=wpar,
                vj_core=vj_core, vi_core=vi_core,
                inv_ls=inv_ls, neg_ls=neg_ls, cf2=cf2)


def kernel(coords, pairs, box, c6, b, cutoff, atom_types):
    coords = np.asarray(coords, np.float32)
    pairs = np.asarray(pairs)
    box = np.asarray(box, np.float32)
    c6 = np.asarray(c6, np.float32)
    b = np.asarray(b, np.float32)
    atom_types = np.asarray(atom_types).astype(np.int64)
    cutoff = float(np.asarray(cutoff))

    offdiag = box - np.diag(np.diag(box))
    diag = np.diag(box)
    if np.any(offdiag != 0.0) or not (diag[0] == diag[1] == diag[2]):
        return _host_reference(coords, pairs, box, c6, b, cutoff, atom_types)

    H = _prepare(coords, pairs, box, c6, b, cutoff, atom_types)
    vj_core, vi_core = H["vj_core"], H["vi_core"]

    # ---- pass 1: expand j-records on device ----
    nc1 = _get_nc(build_pass1, ("p1", vj_core), vj_core)
    in1 = [{"slimv": H["slimv"][c * vj_core:(c + 1) * vj_core]}
           for c in range(N_CORES)]
    res1 = run_bass_kernel_spmd(nc1, in1, core_ids=list(range(N_CORES)))
    recs_A = np.concatenate([res1.results[c]["recs"] for c in range(N_CORES)])

    # ---- host routing: A-slot order -> B-slot order ----
    jrec_B = recs_A[H["a_of_b"]]

    # ---- pass 2: energies per B slot ----
    nc2 = _get_nc(build_pass2,
                  ("p2", vi_core, H["inv_ls"], H["cf2"]),
                  vi_core, H["inv_ls"], H["neg_ls"], H["cf2"])
    sc = vi_core * R_B
    in2 = [dict(wxyz=H["wxyz"][c * vi_core:(c + 1) * vi_core],
                wpar=H["wpar"][c * vi_core:(c + 1) * vi_core],
                jrec=np.ascontiguousarray(jrec_B[c * sc:(c + 1) * sc]))
           for c in range(N_CORES)]
    res2 = run_bass_kernel_spmd(nc2, in2, core_ids=list(range(N_CORES)))
    ene_B = np.concatenate([res2.results[c]["ene"] for c in range(N_CORES)])

    return ene_B[H["sB"]].astype(np.float32)
